# revision 2
# baseline (speedup 1.0000x reference)
"""CPD block (1x1 conv -> depthwise 1x3 -> depthwise 3x1 + bias) on 8 trn2 cores.

Contract: kernel(**inputs) takes FULL inputs (x:[8,64,256,256] f32, w1:[64,64],
wh:[64,3], wv:[64,3], bias:[64]) and returns the FULL output [8,64,256,256] f32.

Strategy
--------
Data-parallel over batch: 1 image per core, 8 cores, no collectives.

The input is zero-padded on the host to [64, 258, 258] f16 and split into two
128-row halves stacked on the 128 SBUF partitions (partition p = 2*c + hh), so
DMA and compute run at full 128-partition width.

The 1x1 conv and the horizontal 1x3 depthwise conv are fused into 3 "tap"
matmuls over the in-channel dim (W_dx[o,c] = w1[o,c]*wh[o,dx]) accumulated in
PSUM; the taps read column-shifted views of the padded x tile.  Each tap
matmul uses K=128 block-diagonal weights (diag(W_dx, W_dx)) so one N=512
instruction computes both halves at once.

The z result is evacuated from PSUM to SBUF in f16 by the scalar engine
(8-row activation casts).  The vertical 3x1 conv + bias runs entirely on the
vector engine using ops that hit DVE 16-bit perf modes (measured on hw):
tensor_scalar at ~424 Gelem/s (4x packing) and tensor_tensor at ~236 Gelem/s
(2x), instead of scalar_tensor_tensor chains at ~118 Gelem/s:
  ua = wv0 * z[r-1]            (TS)
  ub = wv2 * z[r+1]            (TS)
  uc = wv1 * z[r]   + bias     (TS with scalar2-add)
  s  = ua + ub                 (TT)
  ot = s + uc                  (TT)
The output is written to HBM in f16 (halving output traffic) and cast back to
f32 on the host.
"""

import numpy as np

import concourse.bacc as bacc
import concourse.mybir as mybir
from concourse.tile import TileContext
from concourse.bass_utils import run_bass_kernel_spmd

B, C, O = 8, 64, 64
H, W = 256, 256
WP = W + 2             # padded width
N_CORES = 8
HALF = H // 2          # rows per half-image
SEG = 32               # output rows per half per segment
NSEG = HALF // SEG
ZR = SEG + 2           # z rows per segment
ZB = 8                 # z rows per PSUM block (4 banks)

F16 = mybir.dt.float16
F32 = mybir.dt.float32


def _kernel_body(tc, out, x, w, v, hw_reps=0, reps=1, do_mm=True, do_vconv=True,
                 uc_act_rows=24, seg=SEG, xbufs=2, zsbufs=2, vb=32):
    nc = tc.nc
    nseg = HALF // seg
    zr = seg + 2
    mult, add = mybir.AluOpType.mult, mybir.AluOpType.add
    ident = mybir.ActivationFunctionType.Identity

    with (
        tc.tile_pool(name="const", bufs=1) as cpool,
        tc.tile_pool(name="xp", bufs=xbufs) as xpool,
        tc.tile_pool(name="zs", bufs=zsbufs) as zspool,
        tc.tile_pool(name="vt", bufs=1) as vtpool,
        tc.tile_pool(name="op", bufs=2) as opool,
        tc.tile_pool(name="zp", bufs=2, space="PSUM") as zpool,
    ):
        w_sb = cpool.tile([128, 3 * 128], F16)
        nc.sync.dma_start(out=w_sb, in_=w)
        v_sb = cpool.tile([128, 4], F32)
        nc.sync.dma_start(out=v_sb, in_=v)

        # Partition convention: p = 2*c + hh (channel-major, half fastest).
        orr = out.rearrange("c (hh hr) w -> c hh hr w", hh=2)

        def emit_segment(s):
            r0 = s * seg  # segment start row, half-local coords
            # x is host-prepped as [128, HALF+2, WP]: partition p = 2c+hh
            # already carries that half's rows (with halo); one full-width DMA.
            # z row i (tile-local) = half out-row r0-1+i = x tile row i.
            xt = xpool.tile([128, zr, WP], F16, tag="xt")
            nc.sync.dma_start(out=xt, in_=x[:, r0 : r0 + zr, :])

            zseg = zspool.tile([128, zr, W], F16, tag="zseg")

            # 1x1 conv + horizontal conv: 3 taps accumulated in PSUM in
            # ZB-row blocks; ACT casts each block out to f16.
            if do_mm:
                for b0 in range(0, zr, ZB):
                    zb = min(ZB, zr - b0)
                    zt = zpool.tile([128, ZB * W], F32, tag="zt")
                    for i in range(3):  # tap-outer: lhsT fixed across chunks
                        for j in range(zb // 2):
                            xr = b0 + 2 * j
                            nc.tensor.matmul(
                                out=zt[:, j * 512 : (j + 1) * 512],
                                lhsT=w_sb[:, i * 128 : (i + 1) * 128],
                                rhs=xt[:, xr : xr + 2, i : i + W],
                                start=(i == 0),
                                stop=(i == 2),
                            )
                    nc.scalar.activation(
                        out=zseg[:, b0 : b0 + zb, :],
                        in_=zt.rearrange("p (r w) -> p r w", w=W)[:, :zb, :],
                        func=ident,
                        scale=1.0,
                    )

            # Vertical conv + bias on DVE only, in whole-segment ops that
            # engage the 16-bit perf modes.
            ot = opool.tile([128, seg, W], F16, tag="ot")
            if do_vconv:
                # v-conv in vb-row blocks so tiles stay small at seg=64
                for p0 in range(0, seg, vb):
                    ua = vtpool.tile([128, vb, W], F16, tag="ua")
                    ub = vtpool.tile([128, vb, W], F16, tag="ub")
                    uc = vtpool.tile([128, vb, W], F16, tag="uc")
                    sm = vtpool.tile([128, vb, W], F16, tag="sm")
                    nc.vector.tensor_scalar(
                        out=ua, in0=zseg[:, p0 : p0 + vb, :],
                        scalar1=v_sb[:, 0:1], scalar2=None, op0=mult,
                    )
                    nc.vector.tensor_scalar(
                        out=ub, in0=zseg[:, p0 + 2 : p0 + 2 + vb, :],
                        scalar1=v_sb[:, 2:3], scalar2=None, op0=mult,
                    )
                    # center tap + bias: some rows on ACT, rest on DVE
                    ar = min(uc_act_rows, vb)
                    if ar:
                        nc.scalar.activation(
                            out=uc[:, 0:ar, :],
                            in_=zseg[:, p0 + 1 : p0 + 1 + ar, :],
                            func=ident, scale=v_sb[:, 1:2], bias=v_sb[:, 3:4],
                        )
                    if ar < vb:
                        nc.vector.tensor_scalar(
                            out=uc[:, ar:vb, :],
                            in0=zseg[:, p0 + 1 + ar : p0 + 1 + vb, :],
                            scalar1=v_sb[:, 1:2], scalar2=v_sb[:, 3:4],
                            op0=mult, op1=add,
                        )
                    nc.vector.tensor_tensor(out=sm, in0=ua, in1=ub, op=add)
                    nc.vector.tensor_tensor(
                        out=ot[:, p0 : p0 + vb, :], in0=sm, in1=uc, op=add
                    )

            if do_vconv:
                src = ot
            elif do_mm:
                src = zseg[:, 1 : 1 + seg, :]
            else:
                src = xt[:, 1 : 1 + seg, 1 : 1 + W]
            nc.scalar.dma_start(out=orr[:, :, r0 : r0 + seg, :], in_=src)

        if hw_reps:
            with tc.For_i(0, hw_reps):
                for rep in range(reps):
                    for s in range(nseg):
                        emit_segment(s)
        else:
            for rep in range(reps):
                for s in range(nseg):
                    emit_segment(s)


_CACHE = {}


def _build(hw_reps=0, reps=1, do_mm=True, do_vconv=True, uc_act_rows=24,
           seg=SEG, xbufs=2, zsbufs=2, vb=32):
    key = ("nc", hw_reps, reps, do_mm, do_vconv, uc_act_rows, seg, xbufs,
           zsbufs, vb)
    if key in _CACHE:
        return _CACHE[key]
    nc = bacc.Bacc("TRN2", target_bir_lowering=False, debug=False)
    xd = nc.dram_tensor("x", [128, HALF + 2, WP], F16, kind="ExternalInput").ap()
    wd = nc.dram_tensor("w", [128, 3 * 128], F16, kind="ExternalInput").ap()
    vd = nc.dram_tensor("v", [128, 4], F32, kind="ExternalInput").ap()
    od = nc.dram_tensor("out", [C, H, W], F16, kind="ExternalOutput").ap()
    with TileContext(nc) as tc:
        _kernel_body(tc, od, xd, wd, vd, hw_reps=hw_reps, reps=reps, do_mm=do_mm,
                     do_vconv=do_vconv, uc_act_rows=uc_act_rows, seg=seg,
                     xbufs=xbufs, zsbufs=zsbufs, vb=vb)
    nc.compile()
    _CACHE[key] = nc
    return nc


def prep_inputs(x, w1, wh, wv, bias):
    """Host-side input prep shared by kernel() and benchmarks."""
    x = np.asarray(x, dtype=np.float32)
    w1 = np.asarray(w1, dtype=np.float32)
    wh = np.asarray(wh, dtype=np.float32)
    wv = np.asarray(wv, dtype=np.float32)
    bias = np.asarray(bias, dtype=np.float32)

    # Host-side zero pad, then split into two 128-row halves (with one halo
    # row on each side) stacked on the partition axis: [B, 128, HALF+2, WP].
    xpad = np.zeros((B, C, H + 2, WP), np.float16)
    xpad[:, :, 1 : H + 1, 1 : W + 1] = x.astype(np.float16)
    xp = np.empty((B, C, 2, HALF + 2, WP), np.float16)
    for hh in range(2):
        xp[:, :, hh] = xpad[:, :, hh * HALF : hh * HALF + HALF + 2, :]
    xp = xp.reshape(B, 128, HALF + 2, WP)  # partition p = 2*c + hh

    # Fold the horizontal conv into the 1x1 and build K=128 block-diagonal
    # taps: lhsT_dx = diag(W_dx.T, W_dx.T) with W_dx[o,c] = w1[o,c]*wh[o,dx].
    w_np = np.zeros((128, 3 * 128), np.float16)
    for dx in range(3):
        blk = (w1 * wh[:, dx : dx + 1]).T.astype(np.float16)  # [c, o]
        wb = np.zeros((C, 2, O, 2), np.float16)
        wb[:, 0, :, 0] = blk
        wb[:, 1, :, 1] = blk
        w_np[:, dx * 128 : (dx + 1) * 128] = wb.reshape(128, 128)
    # Per-partition vertical-tap weights + bias: [wv0, wv1, wv2, bias]
    v_np = np.stack([wv[:, 0], wv[:, 1], wv[:, 2], bias], axis=1)
    v_np = np.repeat(v_np, 2, axis=0).astype(np.float32)  # p = 2*c + hh
    return xp, w_np, v_np


def sim_feeds(prepped):
    """Core-0 input map for CoreSim (used by sim.py only)."""
    xp, w_np, v_np = prepped
    return {"x": xp[0], "w": w_np, "v": v_np}


def sim_output(sim, inputs):
    """Core-0 full-precision output from a CoreSim run (sim.py only)."""
    return np.asarray(sim.tensor("out")).astype(np.float32)


def kernel(x, w1, wh, wv, bias, _results_out=None):
    xp, w_np, v_np = prep_inputs(x, w1, wh, wv, bias)
    nc = _build()
    in_maps = [{"x": xp[b], "w": w_np, "v": v_np} for b in range(B)]
    res = run_bass_kernel_spmd(nc, in_maps, list(range(N_CORES)))
    if _results_out is not None:
        _results_out.append(res)
    return np.stack(
        [res.results[b]["out"].astype(np.float32) for b in range(B)], axis=0
    )

